# revision 5
# baseline (speedup 1.0000x reference)
"""MoE feed-forward (shared + top-2 of 8 routed experts), expert-parallel
across 8 trn2 cores.

Sharding strategy (per the spec's expert-parallel hint): the stacked expert
weights [E,d,f] are sharded along E — core c owns expert c. Token dispatch/
combine happens at the shard/unshard boundary on the host: while building
per-core inputs, the host runs the (tiny, 0.1% of FLOPs) router
(softmax -> top-2 -> renormalize) and gathers each expert's tokens into that
core's input shard, padded to a uniform capacity CE (SPMD requires one
program). The shared expert is token-parallel: core c also runs the shared
FFN for tokens [512c, 512(c+1)). Unshard scatters the (device-scaled) expert
rows back and sums with the shared rows; each expert's token list has unique
token ids, so fancy-index += is exact.

Device program per core (identical on all 8):
  - routed:  h^T = Wg^T xg^T, u^T = Wu^T xg^T (bf16, fp32 PSUM), g = silu(h)*u
             y = (g @ Wd) * gate_weight   [CE tokens]
  - shared:  same, 512 tokens, no scaling
Activations keep tokens in the free dim for gate/up (h^T layout, stationary
weights are natural [128,128] tiles); the down matmul consumes g^T tiles as
lhsT and emits token-major y for direct DMA-out.
"""

import numpy as np

E = 8          # routed experts
D = 1024       # hidden
F = 1024       # intermediate
B, S = 2, 2048
T = B * S      # 4096 tokens
NCORES = 8
TCS = T // NCORES  # 512 shared-expert tokens per core
P = 128
DK = D // P    # 8 contraction chunks over D
FT = F // P    # 8 f tiles

_CACHE: dict = {}
_ROUTING: dict = {}


def _psum_groups(n):
    out = []
    o = 0
    while o < n:
        g = min(512, n - o)
        out.append((o, g))
        o += g
    return out


def _build_nc(ce, reps=1, loop_reps=0):
    import concourse.mybir as mybir
    import concourse.tile as tile
    from concourse import bacc
    from concourse.bass import ts, ds

    dt = mybir.dt
    f32 = dt.float32
    bf16 = dt.bfloat16
    Alu = mybir.AluOpType
    Act = mybir.ActivationFunctionType

    nch_r = ce // P           # routed token chunks
    nch = nch_r + TCS // P    # + shared chunks

    nc = bacc.Bacc("TRN2", target_bir_lowering=False, debug=False,
                   num_devices=NCORES)

    xr_d = nc.dram_tensor("xrT", [P, DK, ce], bf16, kind="ExternalInput").ap()
    xs_d = nc.dram_tensor("xsT", [P, DK, TCS], bf16, kind="ExternalInput").ap()
    wt_d = nc.dram_tensor("wrt", [P, nch_r], f32, kind="ExternalInput").ap()
    wg_d = nc.dram_tensor("wg", [2, P, DK, F], bf16, kind="ExternalInput").ap()
    wu_d = nc.dram_tensor("wu", [2, P, DK, F], bf16, kind="ExternalInput").ap()
    wd_d = nc.dram_tensor("wd", [2, P, FT, D], bf16, kind="ExternalInput").ap()
    y_d = nc.dram_tensor("y", [nch, P, D], bf16, kind="ExternalOutput").ap()

    with tile.TileContext(nc) as tc:
        with (
            tc.tile_pool(name="const", bufs=1) as constp,
            tc.tile_pool(name="wgp", bufs=2) as wgp,
            tc.tile_pool(name="wup", bufs=2) as wup,
            tc.tile_pool(name="wdp", bufs=2) as wdp,
            tc.tile_pool(name="gp", bufs=2) as gp,
            tc.tile_pool(name="yp", bufs=4) as yp,
            tc.tile_pool(name="php", bufs=6, space="PSUM") as php,
            tc.tile_pool(name="pyp", bufs=2, space="PSUM") as pyp,
        ):
          import contextlib
          loop_cm = (tc.For_i(0, loop_reps, 1) if loop_reps
                     else contextlib.nullcontext())
          with loop_cm:
           for _rep in range(reps):
              xr = constp.tile([P, DK, ce], bf16)
              for dk in range(DK):
                  nc.sync.dma_start(xr[:, dk], xr_d[:, dk])
              xs = constp.tile([P, DK, TCS], bf16)
              for dk in range(DK):
                  nc.sync.dma_start(xs[:, dk], xs_d[:, dk])
              wtok = constp.tile([P, nch_r], f32)
              nc.sync.dma_start(wtok[:], wt_d[:])

              def ffn(e, xa, ntok, chunk0, scaled):
                  # weights for this part (split DMAs to spread across both
                  # HWDGE rings: wg/wd on SP, wu on ACT)
                  wg_sb = wgp.tile([P, DK, F], bf16, tag="wg")
                  for dk in range(DK):
                      nc.sync.dma_start(wg_sb[:, dk], wg_d[e, :, dk])
                  wu_sb = wup.tile([P, DK, F], bf16, tag="wu")
                  for dk in range(DK):
                      nc.sync.dma_start(wu_sb[:, dk], wu_d[e, :, dk])
                  wd_sb = wdp.tile([P, FT, D], bf16, tag="wd")
                  for fk in range(FT):
                      nc.sync.dma_start(wd_sb[:, fk], wd_d[e, :, fk])

                  g_sb = gp.tile([P, FT, ntok], bf16, tag="g")
                  groups = _psum_groups(ntok)
                  for ft in range(FT):
                      # dk outer: each [128,128] weight tile streams all ntok
                      # rows (across psum groups) back-to-back
                      phs = []
                      for gi in range(len(groups)):
                          ph = php.tile([P, groups[gi][1]], f32, tag="ph")
                          phs.append(ph)
                      for dk in range(DK):
                          for gi, (go, gl) in enumerate(groups):
                              nc.tensor.matmul(
                                  phs[gi][:], wg_sb[:, dk, ts(ft, P)],
                                  xa[:, dk, ds(go, gl)],
                                  start=(dk == 0), stop=(dk == DK - 1),
                              )
                      pus = []
                      for gi in range(len(groups)):
                          pu = php.tile([P, groups[gi][1]], f32, tag="ph")
                          pus.append(pu)
                      for dk in range(DK):
                          for gi, (go, gl) in enumerate(groups):
                              nc.tensor.matmul(
                                  pus[gi][:], wu_sb[:, dk, ts(ft, P)],
                                  xa[:, dk, ds(go, gl)],
                                  start=(dk == 0), stop=(dk == DK - 1),
                              )
                      for gi, (go, gl) in enumerate(groups):
                          nc.scalar.activation(g_sb[:, ft, ds(go, gl)],
                                               phs[gi][:], Act.Silu)
                          nc.vector.tensor_tensor(g_sb[:, ft, ds(go, gl)],
                                                  g_sb[:, ft, ds(go, gl)],
                                                  pus[gi][:], Alu.mult)

                  for tch in range(ntok // P):
                      # dh inner so each g lhsT tile streams both 512-halves
                      py0 = pyp.tile([P, 512], f32, tag="py")
                      py1 = pyp.tile([P, 512], f32, tag="py")
                      pys = [py0, py1]
                      for fk in range(FT):
                          for dh in range(2):
                              nc.tensor.matmul(
                                  pys[dh][:], g_sb[:, fk, ts(tch, P)],
                                  wd_sb[:, fk, ds(dh * 512, 512)],
                                  start=(fk == 0), stop=(fk == FT - 1),
                              )
                      for dh in range(2):
                          ysb = yp.tile([P, 512], bf16, tag="y")
                          if scaled:
                              nc.vector.tensor_scalar(
                                  ysb[:], pys[dh][:], wtok[:, tch:tch + 1],
                                  None, op0=Alu.mult)
                          else:
                              nc.vector.tensor_copy(ysb[:], pys[dh][:])
                          nc.scalar.dma_start(
                              y_d[chunk0 + tch, :, ds(dh * 512, 512)], ysb[:])

              ffn(0, xr, ce, 0, True)
              ffn(1, xs, TCS, ce // P, False)

    nc.compile()
    return nc


def _get_nc(reps=1, loop_reps=0):
    ce = _ROUTING["ce"]
    key = (ce, reps, loop_reps)
    if key not in _CACHE:
        _CACHE[key] = _build_nc(ce, reps, loop_reps)
    return _CACHE[key]


def _route(x, gate_w):
    """Host router: softmax -> top-2 (jax top_k tie order) -> renormalize."""
    xf = np.asarray(x, np.float32).reshape(T, D)
    logits = xf @ np.asarray(gate_w, np.float32)
    m = logits.max(-1, keepdims=True)
    q = np.exp(logits - m)
    gate = q / q.sum(-1, keepdims=True)
    order = np.argsort(-gate, axis=-1, kind="stable")
    topi = order[:, :2]
    topw = np.take_along_axis(gate, topi, axis=-1)
    topw = topw / (topw.sum(-1, keepdims=True) + 1e-20)
    return xf, topi, topw


def make_in_maps(x, gate_w, sw_gate, sw_up, sw_down, ew_gate, ew_up, ew_down):
    import ml_dtypes
    bf16 = ml_dtypes.bfloat16

    xf, topi, topw = _route(x, gate_w)

    idxs, ws = [], []
    for e in range(E):
        sel = np.nonzero(topi == e)
        idxs.append(sel[0].astype(np.int64))          # token ids, sorted
        ws.append(topw[sel].astype(np.float32))
    counts = [len(i) for i in idxs]
    ce = ((max(counts) + P - 1) // P) * P
    _ROUTING.update(ce=ce, idxs=idxs, counts=counts)

    sw = [np.asarray(a, np.float32) for a in (sw_gate, sw_up, sw_down)]
    ew = [np.asarray(a, np.float32) for a in (ew_gate, ew_up, ew_down)]

    def prep_gu(w):   # [D, F] -> [128, DK, F] bf16
        return np.ascontiguousarray(
            w.reshape(DK, P, F).transpose(1, 0, 2).astype(bf16))

    def prep_d(w):    # [F, D] -> [128, FT, D] bf16
        return np.ascontiguousarray(
            w.reshape(FT, P, D).transpose(1, 0, 2).astype(bf16))

    def prep_x(rows, n):  # [n?, D] pad to n -> [128, DK, n] bf16
        xp = np.zeros((n, D), np.float32)
        xp[:len(rows)] = rows
        return np.ascontiguousarray(
            xp.T.reshape(DK, P, n).transpose(1, 0, 2).astype(bf16))

    sg, su, sd = (prep_gu(sw[0]), prep_gu(sw[1]), prep_d(sw[2]))

    in_maps = []
    for c in range(NCORES):
        xg = prep_x(xf[idxs[c]], ce)
        xsl = prep_x(xf[c * TCS:(c + 1) * TCS], TCS)
        wt = np.zeros(ce, np.float32)
        wt[:counts[c]] = ws[c]
        wt = np.ascontiguousarray(wt.reshape(ce // P, P).T)   # [128, nch_r]
        in_maps.append({
            "xrT": xg, "xsT": xsl, "wrt": wt,
            "wg": np.stack([prep_gu(ew[0][c]), sg]),
            "wu": np.stack([prep_gu(ew[1][c]), su]),
            "wd": np.stack([prep_d(ew[2][c]), sd]),
        })
    return in_maps


def assemble_out(results):
    ce = _ROUTING["ce"]
    idxs, counts = _ROUTING["idxs"], _ROUTING["counts"]
    y = np.zeros((T, D), np.float32)
    for c in range(NCORES):
        yr = np.asarray(results[c]["y"], np.float32).reshape(-1, D)
        y[c * TCS:(c + 1) * TCS] = yr[ce:ce + TCS]
    for c in range(NCORES):
        yr = np.asarray(results[c]["y"], np.float32).reshape(-1, D)
        y[idxs[c]] += yr[:counts[c]]
    return y.reshape(B, S, D)


def kernel(x, gate_w, sw_gate, sw_up, sw_down, ew_gate, ew_up, ew_down):
    from concourse.bass_utils import run_bass_kernel_spmd

    in_maps = make_in_maps(x, gate_w, sw_gate, sw_up, sw_down,
                           ew_gate, ew_up, ew_down)
    nc = _get_nc()
    res = run_bass_kernel_spmd(nc, in_maps, list(range(NCORES)))
    return assemble_out(res.results)


# revision 7
# speedup vs baseline: 1.0492x; 1.0492x over previous
"""MoE feed-forward (shared + top-2 of 8 routed experts), expert-parallel
across 8 trn2 cores.

Sharding strategy (per the spec's expert-parallel hint): the stacked expert
weights [E,d,f] are sharded along E — core c owns expert c. Token dispatch/
combine happens at the shard/unshard boundary on the host: while building
per-core inputs, the host runs the (tiny, 0.1% of FLOPs) router
(softmax -> top-2 -> renormalize) and gathers each expert's tokens into that
core's input shard, padded to a uniform capacity CE (SPMD requires one
program). The shared expert is token-parallel: core c also runs the shared
FFN for tokens [512c, 512(c+1)). Unshard scatters the (device-scaled) expert
rows back and sums with the shared rows; each expert's token list has unique
token ids, so fancy-index += is exact.

Device program per core (identical on all 8):
  - routed:  h^T = Wg^T xg^T, u^T = Wu^T xg^T (bf16, fp32 PSUM), g = silu(h)*u
             y = (g @ Wd) * gate_weight   [CE tokens]
  - shared:  same, 512 tokens, no scaling
Activations keep tokens in the free dim for gate/up (h^T layout, stationary
weights are natural [128,128] tiles); the down matmul consumes g^T tiles as
lhsT and emits token-major y for direct DMA-out.
"""

import numpy as np

E = 8          # routed experts
D = 1024       # hidden
F = 1024       # intermediate
B, S = 2, 2048
T = B * S      # 4096 tokens
NCORES = 8
TCS = T // NCORES  # 512 shared-expert tokens per core
P = 128
DK = D // P    # 8 contraction chunks over D
FT = F // P    # 8 f tiles

_CACHE: dict = {}
_ROUTING: dict = {}


def _psum_groups(n):
    # prefer uniform 384-token groups (weight-load 107ns hides fully under
    # a 160ns stream); fall back to 512-chunks
    if n > 512 and n % 384 == 0:
        return [(i * 384, 384) for i in range(n // 384)]
    out = []
    o = 0
    while o < n:
        g = min(512, n - o)
        out.append((o, g))
        o += g
    return out


def _build_nc(ce, reps=1, loop_reps=0):
    import concourse.mybir as mybir
    import concourse.tile as tile
    from concourse import bacc
    from concourse.bass import ts, ds

    dt = mybir.dt
    f32 = dt.float32
    bf16 = dt.bfloat16
    Alu = mybir.AluOpType
    Act = mybir.ActivationFunctionType

    nch_r = ce // P           # routed token chunks
    nch = nch_r + TCS // P    # + shared chunks

    nc = bacc.Bacc("TRN2", target_bir_lowering=False, debug=False,
                   num_devices=NCORES)

    xr_d = nc.dram_tensor("xrT", [P, DK, ce], bf16, kind="ExternalInput").ap()
    xs_d = nc.dram_tensor("xsT", [P, DK, TCS], bf16, kind="ExternalInput").ap()
    wt_d = nc.dram_tensor("wrt", [P, nch_r], f32, kind="ExternalInput").ap()
    wg_d = nc.dram_tensor("wg", [2, P, DK, F], bf16, kind="ExternalInput").ap()
    wu_d = nc.dram_tensor("wu", [2, P, DK, F], bf16, kind="ExternalInput").ap()
    wd_d = nc.dram_tensor("wd", [2, P, FT, D], bf16, kind="ExternalInput").ap()
    y_d = nc.dram_tensor("y", [nch, P, D], bf16, kind="ExternalOutput").ap()

    with tile.TileContext(nc) as tc:
        with (
            tc.tile_pool(name="const", bufs=1) as constp,
            tc.tile_pool(name="wgp", bufs=2) as wgp,
            tc.tile_pool(name="wup", bufs=2) as wup,
            tc.tile_pool(name="wdp", bufs=2) as wdp,
            tc.tile_pool(name="gp", bufs=2) as gp,
            tc.tile_pool(name="yp", bufs=4) as yp,
            tc.tile_pool(name="php", bufs=8, space="PSUM") as php,
        ):
          import contextlib
          loop_cm = (tc.For_i(0, loop_reps, 1) if loop_reps
                     else contextlib.nullcontext())
          with loop_cm:
           for _rep in range(reps):
              xr = constp.tile([P, DK, ce], bf16)
              for dk in range(DK):
                  nc.sync.dma_start(xr[:, dk], xr_d[:, dk])
              xs = constp.tile([P, DK, TCS], bf16)
              for dk in range(DK):
                  nc.sync.dma_start(xs[:, dk], xs_d[:, dk])
              wtok = constp.tile([P, nch_r], f32)
              nc.sync.dma_start(wtok[:], wt_d[:])

              def ffn(e, xa, ntok, chunk0, scaled):
                  # weights for this part (split DMAs to spread across both
                  # HWDGE rings: wg/wd on SP, wu on ACT)
                  wg_sb = wgp.tile([P, DK, F], bf16, tag="wg")
                  for dk in range(DK):
                      nc.sync.dma_start(wg_sb[:, dk], wg_d[e, :, dk])
                  wu_sb = wup.tile([P, DK, F], bf16, tag="wu")
                  for dk in range(DK):
                      nc.sync.dma_start(wu_sb[:, dk], wu_d[e, :, dk])
                  wd_sb = wdp.tile([P, FT, D], bf16, tag="wd")
                  for fk in range(FT):
                      nc.sync.dma_start(wd_sb[:, fk], wd_d[e, :, fk])

                  g_sb = gp.tile([P, FT, ntok], bf16, tag="g")
                  groups = _psum_groups(ntok)
                  for ft in range(FT):
                      # dk outer: each [128,128] weight tile streams all ntok
                      # rows (across psum groups) back-to-back
                      phs = []
                      for gi in range(len(groups)):
                          ph = php.tile([P, groups[gi][1]], f32, tag="ph")
                          phs.append(ph)
                      for dk in range(DK):
                          for gi, (go, gl) in enumerate(groups):
                              nc.tensor.matmul(
                                  phs[gi][:], wg_sb[:, dk, ts(ft, P)],
                                  xa[:, dk, ds(go, gl)],
                                  start=(dk == 0), stop=(dk == DK - 1),
                              )
                      pus = []
                      for gi in range(len(groups)):
                          pu = php.tile([P, groups[gi][1]], f32, tag="ph")
                          pus.append(pu)
                      for dk in range(DK):
                          for gi, (go, gl) in enumerate(groups):
                              nc.tensor.matmul(
                                  pus[gi][:], wu_sb[:, dk, ts(ft, P)],
                                  xa[:, dk, ds(go, gl)],
                                  start=(dk == 0), stop=(dk == DK - 1),
                              )
                      for gi, (go, gl) in enumerate(groups):
                          nc.scalar.activation(g_sb[:, ft, ds(go, gl)],
                                               phs[gi][:], Act.Silu)
                          nc.vector.tensor_tensor(g_sb[:, ft, ds(go, gl)],
                                                  g_sb[:, ft, ds(go, gl)],
                                                  pus[gi][:], Alu.mult)

                  for tch in range(ntok // P):
                      # dh inner so each g lhsT tile streams both 512-halves
                      py0 = php.tile([P, 512], f32, tag="ph")
                      py1 = php.tile([P, 512], f32, tag="ph")
                      pys = [py0, py1]
                      for fk in range(FT):
                          for dh in range(2):
                              nc.tensor.matmul(
                                  pys[dh][:], g_sb[:, fk, ts(tch, P)],
                                  wd_sb[:, fk, ds(dh * 512, 512)],
                                  start=(fk == 0), stop=(fk == FT - 1),
                              )
                      for dh in range(2):
                          ysb = yp.tile([P, 512], bf16, tag="y")
                          if scaled:
                              nc.vector.tensor_scalar(
                                  ysb[:], pys[dh][:], wtok[:, tch:tch + 1],
                                  None, op0=Alu.mult)
                          else:
                              nc.vector.tensor_copy(ysb[:], pys[dh][:])
                          nc.sync.dma_start(
                              y_d[chunk0 + tch, :, ds(dh * 512, 512)], ysb[:])

              ffn(0, xr, ce, 0, True)
              ffn(1, xs, TCS, ce // P, False)

    nc.compile()
    return nc


def _get_nc(reps=1, loop_reps=0):
    ce = _ROUTING["ce"]
    key = (ce, reps, loop_reps)
    if key not in _CACHE:
        _CACHE[key] = _build_nc(ce, reps, loop_reps)
    return _CACHE[key]


def _route(x, gate_w):
    """Host router: softmax -> top-2 (jax top_k tie order) -> renormalize."""
    xf = np.asarray(x, np.float32).reshape(T, D)
    logits = xf @ np.asarray(gate_w, np.float32)
    m = logits.max(-1, keepdims=True)
    q = np.exp(logits - m)
    gate = q / q.sum(-1, keepdims=True)
    order = np.argsort(-gate, axis=-1, kind="stable")
    topi = order[:, :2]
    topw = np.take_along_axis(gate, topi, axis=-1)
    topw = topw / (topw.sum(-1, keepdims=True) + 1e-20)
    return xf, topi, topw


def make_in_maps(x, gate_w, sw_gate, sw_up, sw_down, ew_gate, ew_up, ew_down):
    import ml_dtypes
    bf16 = ml_dtypes.bfloat16

    xf, topi, topw = _route(x, gate_w)

    idxs, ws = [], []
    for e in range(E):
        sel = np.nonzero(topi == e)
        idxs.append(sel[0].astype(np.int64))          # token ids, sorted
        ws.append(topw[sel].astype(np.float32))
    counts = [len(i) for i in idxs]
    ce = ((max(counts) + P - 1) // P) * P
    _ROUTING.update(ce=ce, idxs=idxs, counts=counts)

    sw = [np.asarray(a, np.float32) for a in (sw_gate, sw_up, sw_down)]
    ew = [np.asarray(a, np.float32) for a in (ew_gate, ew_up, ew_down)]

    def prep_gu(w):   # [D, F] -> [128, DK, F] bf16
        return np.ascontiguousarray(
            w.reshape(DK, P, F).transpose(1, 0, 2).astype(bf16))

    def prep_d(w):    # [F, D] -> [128, FT, D] bf16
        return np.ascontiguousarray(
            w.reshape(FT, P, D).transpose(1, 0, 2).astype(bf16))

    def prep_x(rows, n):  # [n?, D] pad to n -> [128, DK, n] bf16
        xp = np.zeros((n, D), np.float32)
        xp[:len(rows)] = rows
        return np.ascontiguousarray(
            xp.T.reshape(DK, P, n).transpose(1, 0, 2).astype(bf16))

    sg, su, sd = (prep_gu(sw[0]), prep_gu(sw[1]), prep_d(sw[2]))

    in_maps = []
    for c in range(NCORES):
        xg = prep_x(xf[idxs[c]], ce)
        xsl = prep_x(xf[c * TCS:(c + 1) * TCS], TCS)
        wt = np.zeros(ce, np.float32)
        wt[:counts[c]] = ws[c]
        wt = np.ascontiguousarray(wt.reshape(ce // P, P).T)   # [128, nch_r]
        in_maps.append({
            "xrT": xg, "xsT": xsl, "wrt": wt,
            "wg": np.stack([prep_gu(ew[0][c]), sg]),
            "wu": np.stack([prep_gu(ew[1][c]), su]),
            "wd": np.stack([prep_d(ew[2][c]), sd]),
        })
    return in_maps


def assemble_out(results):
    ce = _ROUTING["ce"]
    idxs, counts = _ROUTING["idxs"], _ROUTING["counts"]
    y = np.zeros((T, D), np.float32)
    for c in range(NCORES):
        yr = np.asarray(results[c]["y"], np.float32).reshape(-1, D)
        y[c * TCS:(c + 1) * TCS] = yr[ce:ce + TCS]
    for c in range(NCORES):
        yr = np.asarray(results[c]["y"], np.float32).reshape(-1, D)
        y[idxs[c]] += yr[:counts[c]]
    return y.reshape(B, S, D)


def kernel(x, gate_w, sw_gate, sw_up, sw_down, ew_gate, ew_up, ew_down):
    from concourse.bass_utils import run_bass_kernel_spmd

    in_maps = make_in_maps(x, gate_w, sw_gate, sw_up, sw_down,
                           ew_gate, ew_up, ew_down)
    nc = _get_nc()
    res = run_bass_kernel_spmd(nc, in_maps, list(range(NCORES)))
    return assemble_out(res.results)
